# revision 4
# baseline (speedup 1.0000x reference)
"""MoE FFN (8 routed experts, top-2, + shared expert) on 8 TRN2 NeuronCores.

Sharding (per the expert-parallel hint):
  - Routed experts: expert-parallel. Core e owns expert e. The gate (tiny
    8192x8 matmul + top-2 softmax) is computed on host; each expert's tokens
    are gathered on host into a fixed-capacity buffer, the device runs the
    dense expert FFN on them, and the host scatter-adds the results back with
    the gate weights.
  - Shared expert: tensor-parallel column split. Core e holds Sg/Su columns
    [e*1024:(e+1)*1024] and Sd rows [e*1024:(e+1)*1024]; every core processes
    all tokens and produces a full-size partial that the host sums.

Device layout: all matmuls run with features on the partition dim and tokens
on the free dim, so every weight loads in its natural layout and x is
transposed once on the host.  Matmuls run as float32r (full PE rate,
~1.5e-4 matmul rel err vs ~2.4e-3 for bf16).
"""

import numpy as np

import concourse.bass as bass
import concourse.mybir as mybir
import concourse.tile as tile
from concourse import bacc
from concourse.bass_utils import run_bass_kernel_spmd

D_MODEL = 2048
N_ROUTED = 8
TOP_K = 2
H_ROUTED = 1024
H_SHARED = 8192
N_TOK = 8192
N_CORES = 8

CAP = 2560          # per-expert token capacity (multiple of 512)
TB = 1024           # token block
NT = 512            # matmul free dim / PSUM bank width
P = 128

F32 = mybir.dt.float32
R32 = mybir.dt.float32r


def _ffn_pass(nc, pools, psum, x_dram, wg_dram, wu_dram, wd_dram, out_dram, T):
    """out_dram[D, T] = Wd.T @ (silu(Wg.T @ x) * (Wu.T @ x)) for x = x_dram[D, T].

    wg_dram/wu_dram: [D_MODEL, H]; wd_dram: [H, D_MODEL]. H = wg_dram.shape[1].
    """
    H = wg_dram.shape[1]
    KC = D_MODEL // P          # contraction chunks for the up projections
    MH = H // P                # h-dim chunks
    MD = D_MODEL // P          # output d chunks
    silu = mybir.ActivationFunctionType.Silu

    wg_r = wg_dram.rearrange("(c p) h -> p c h", p=P)
    wu_r = wu_dram.rearrange("(c p) h -> p c h", p=P)
    wd_r = wd_dram.rearrange("(c p) d -> p c d", p=P)

    # token blocks of TB, last block may be shorter (multiple of NT)
    blocks = []
    t0 = 0
    while t0 < T:
        tb = min(TB, T - t0)
        blocks.append((t0, tb))
        t0 += tb

    for (t0, tb) in blocks:
        nnt = tb // NT
        xtb = []
        for k in range(KC):
            t = pools["xtb"].tile([P, tb], R32, tag="xtb")
            nc.sync.dma_start(t[:], x_dram[k * P:(k + 1) * P, t0:t0 + tb])
            xtb.append(t)

        hs = []
        for mh in range(MH):
            wgm = pools["w"].tile([P, KC, P], R32, tag="wgm")
            nc.sync.dma_start(wgm[:], wg_r[:, :, mh * P:(mh + 1) * P])
            wum = pools["w"].tile([P, KC, P], R32, tag="wum")
            nc.sync.dma_start(wum[:], wu_r[:, :, mh * P:(mh + 1) * P])
            hm = pools["h"].tile([P, tb], R32, tag="h")
            for nt in range(nnt):
                ts = slice(nt * NT, (nt + 1) * NT)
                psg = psum.tile([P, NT], F32, tag="psg")
                psu = psum.tile([P, NT], F32, tag="psu")
                for k in range(KC):
                    nc.tensor.matmul(
                        psg[:], wgm[:, k, :], xtb[k][:, ts],
                        start=(k == 0), stop=(k == KC - 1),
                    )
                for k in range(KC):
                    nc.tensor.matmul(
                        psu[:], wum[:, k, :], xtb[k][:, ts],
                        start=(k == 0), stop=(k == KC - 1),
                    )
                sl = pools["small"].tile([P, NT], F32, tag="sl")
                nc.scalar.activation(sl[:], psg[:], silu)
                nc.vector.tensor_mul(hm[:, ts], sl[:], psu[:])
            hs.append(hm)

        for md in range(MD):
            wdm = pools["w"].tile([P, MH, P], R32, tag="wdm")
            nc.sync.dma_start(wdm[:], wd_r[:, :, md * P:(md + 1) * P])
            for nt in range(nnt):
                ts = slice(nt * NT, (nt + 1) * NT)
                pso = psum.tile([P, NT], F32, tag="pso")
                for kh in range(MH):
                    nc.tensor.matmul(
                        pso[:], wdm[:, kh, :], hs[kh][:, ts],
                        start=(kh == 0), stop=(kh == MH - 1),
                    )
                ot = pools["small"].tile([P, NT], F32, tag="ot")
                nc.vector.tensor_copy(ot[:], pso[:])
                nc.sync.dma_start(
                    out_dram[md * P:(md + 1) * P, t0 + nt * NT:t0 + (nt + 1) * NT],
                    ot[:],
                )


def _build(cap):
    nc = bacc.Bacc(None, target_bir_lowering=False)
    hr = H_SHARED // N_CORES  # per-core shared column slice == 1024 == H_ROUTED

    xt = nc.dram_tensor("xt", [D_MODEL, N_TOK], R32, kind="ExternalInput")
    xg = nc.dram_tensor("xg", [D_MODEL, cap], R32, kind="ExternalInput")
    sg = nc.dram_tensor("sg", [D_MODEL, hr], R32, kind="ExternalInput")
    su = nc.dram_tensor("su", [D_MODEL, hr], R32, kind="ExternalInput")
    sd = nc.dram_tensor("sd", [hr, D_MODEL], R32, kind="ExternalInput")
    wg = nc.dram_tensor("wg", [D_MODEL, H_ROUTED], R32, kind="ExternalInput")
    wu = nc.dram_tensor("wu", [D_MODEL, H_ROUTED], R32, kind="ExternalInput")
    wd = nc.dram_tensor("wd", [H_ROUTED, D_MODEL], R32, kind="ExternalInput")
    outs = nc.dram_tensor("outs", [D_MODEL, N_TOK], F32, kind="ExternalOutput")
    outr = nc.dram_tensor("outr", [D_MODEL, cap], F32, kind="ExternalOutput")

    with tile.TileContext(nc) as tc:
        with (
            tc.tile_pool(name="xtb", bufs=17) as p_xtb,
            tc.tile_pool(name="h", bufs=9) as p_h,
            tc.tile_pool(name="w", bufs=3) as p_w,
            tc.tile_pool(name="small", bufs=4) as p_small,
            tc.tile_pool(name="psum", bufs=2, space="PSUM") as psum,
        ):
            pools = {"xtb": p_xtb, "h": p_h, "w": p_w, "small": p_small}
            _ffn_pass(nc, pools, psum, xt, sg, su, sd, outs, N_TOK)
            _ffn_pass(nc, pools, psum, xg, wg, wu, wd, outr, cap)
    nc.finalize()
    return nc


def _gate(xf, w_gate):
    """numpy replica of the reference gate. Returns (combine [N,E], lb_loss)."""
    clean = xf @ w_gate.T                                   # [N, E]
    order = np.argsort(-clean, axis=-1, kind="stable")
    top_idx = order[:, :TOP_K]                              # [N, K]
    top_logits = np.take_along_axis(clean, top_idx, axis=1)
    m = top_logits.max(axis=1, keepdims=True)
    g = np.exp(top_logits - m)
    g = g / g.sum(axis=1, keepdims=True)
    combine = np.zeros((xf.shape[0], N_ROUTED), np.float32)
    np.put_along_axis(combine, top_idx, g.astype(np.float32), axis=1)

    mm = clean.max(axis=1, keepdims=True)
    p = np.exp(clean - mm)
    p = p / p.sum(axis=1, keepdims=True)
    importance = p.sum(0)
    ce = importance / xf.shape[0] * N_ROUTED
    lb_loss = np.float32(np.mean(ce * ce))
    return combine, lb_loss


def kernel(x, w_gate, Wg, Wu, Wd, Sg, Su, Sd):
    x = np.asarray(x, np.float32)
    w_gate = np.asarray(w_gate, np.float32)
    Wg = np.asarray(Wg, np.float32)
    Wu = np.asarray(Wu, np.float32)
    Wd = np.asarray(Wd, np.float32)
    Sg = np.asarray(Sg, np.float32)
    Su = np.asarray(Su, np.float32)
    Sd = np.asarray(Sd, np.float32)

    B, T, D = x.shape
    N = B * T
    xf = x.reshape(N, D)

    combine, lb_loss = _gate(xf, w_gate)
    xt = np.ascontiguousarray(xf.T)                          # [D, N]

    idx = [np.nonzero(combine[:, e] > 0)[0] for e in range(N_ROUTED)]
    maxc = max(len(i) for i in idx)
    cap = CAP
    while cap < maxc:
        cap += 512

    hr = H_SHARED // N_CORES
    in_maps = []
    for e in range(N_CORES):
        xg = np.zeros((D, cap), np.float32)
        xg[:, :len(idx[e])] = xt[:, idx[e]]
        in_maps.append({
            "xt": xt,
            "xg": xg,
            "sg": np.ascontiguousarray(Sg[:, e * hr:(e + 1) * hr]),
            "su": np.ascontiguousarray(Su[:, e * hr:(e + 1) * hr]),
            "sd": np.ascontiguousarray(Sd[e * hr:(e + 1) * hr, :]),
            "wg": np.ascontiguousarray(Wg[e]),
            "wu": np.ascontiguousarray(Wu[e]),
            "wd": np.ascontiguousarray(Wd[e]),
        })

    nc = _build(cap)
    res = run_bass_kernel_spmd(nc, in_maps, list(range(N_CORES)))

    acc = res.results[0]["outs"].astype(np.float32)
    for e in range(1, N_CORES):
        acc += res.results[e]["outs"]
    for e in range(N_CORES):
        ie = idx[e]
        acc[:, ie] += res.results[e]["outr"][:, :len(ie)] * combine[ie, e][None, :]

    out = np.ascontiguousarray(acc.T).reshape(B, T, D).astype(np.float32)
    return out, lb_loss
